# revision 8
# baseline (speedup 1.0000x reference)
"""Trainium2 Bass kernel for nn_Correlation (81-displacement cost volume).

corr(b, d, y, x) = sum_c f1[b,c,y,x] * f2[b,c,y+dy,x+dx],  d = (dy+4)*9 + (dx+4)

Sharding: data-parallel over batch B=8, one batch per NeuronCore.

Per-core algorithm (all matmuls bf16, PSUM fp32):
  Tile the (y, x) output plane into blocks of G=16 y-rows x A=8 x-cols.
  For block (g, cc) the PE computes, per channel-half ch (K=128 each):
      psum[m=(s,xi), n=(row,xw)] += f1[c, y=16g+s, x=8cc+xi] *
                                    f2p[c, yp=16g+row, xp=8cc+xw]
  with s in [0,16), xi in [0,8)  (M = 128 weights = one f1 block)
  and row in [0,24), xw in [0,16) (N = 384 = the 24x16 padded f2 window).
  Entry (s,xi,row,xw) equals corr(y=16g+s, x=8cc+xi, dy=row-s, dx=xw-xi)
  - every (dy,dx) in [0,9)^2 is present.  PE cost: 4*16*2*384 = 49k cols
  (vs 157k for the per-row band scheme).
  DVE/ACT alternate evacuating psum -> bf16 stage tile [128, 64*384].
  The per-partition shear is only within each 16-partition group k
  (p = 8s+xi, k = s//2): the needed elements of tile t lie in the
  CONTIGUOUS slice [32k, 32k+160) of that tile's 384 columns, so plain
  DMA slabs (one per 16-partition group) ship them; the fine per-lane
  gather e = (s%2+dy)*16 + xi + dx happens on host.
  f2 zero-padding (4 rows/cols each side) is done on-device via memsets;
  only the real 64x128 f2 data is DMAed.
"""

import sys

sys.path.insert(0, "/opt/trn_rl_repo")

from contextlib import ExitStack

import ml_dtypes
import numpy as np

import jax

jax.config.update("jax_compilation_cache_dir", "/root/jaxcache")
jax.config.update("jax_persistent_cache_min_entry_size_bytes", 0)
jax.config.update("jax_persistent_cache_min_compile_time_secs", 0)

import concourse.bass as bass
import concourse.tile as tile
from concourse import bacc, mybir
from concourse.bass_utils import run_bass_kernel_spmd

F32 = mybir.dt.float32
BF16 = mybir.dt.bfloat16
BF16_NP = ml_dtypes.bfloat16

B = 8
C = 256
H = 64
W = 128
PAD = 4
G = 16       # y rows per block
A = 8        # x cols per block
NG = H // G  # 4 y-blocks
NC_ = W // A  # 16 x-blocks
ROWS = G + 2 * PAD   # 24 padded rows per block window
WIN = A + 2 * PAD    # 16 padded cols per block window
NMM = ROWS * WIN     # 384 psum columns per block
HP = H + 2 * PAD     # 72
WP = W + 2 * PAD     # 136
NB = 81
SLAB = (2 * PAD + 2) * WIN  # 160: contiguous slice per 16-partition group


def build_program():
    nc = bacc.Bacc("TRN2", target_bir_lowering=False, debug=False)

    f1r_d = nc.dram_tensor("f1r", [NG, 128, 2 * G * W], BF16, kind="ExternalInput").ap()
    f2_d = nc.dram_tensor("f2", [128, 2, H, W], BF16, kind="ExternalInput").ap()
    s1_d = nc.dram_tensor("s1", [128, NG * NC_, SLAB], BF16, kind="ExternalOutput").ap()

    # real-row chunks, in the order the y-blocks consume them
    # padded rows [16g, 16g+24) = real rows [16g-4, 16g+20)
    CHUNKS = [(0, 20), (20, 36), (36, 52), (52, 64)]

    with tile.TileContext(nc) as tc, ExitStack() as ctx:
        f2_pool = ctx.enter_context(tc.tile_pool(name="f2", bufs=1))
        f1_pool = ctx.enter_context(tc.tile_pool(name="f1", bufs=NG))
        stage_pool = ctx.enter_context(tc.tile_pool(name="stage", bufs=1))
        psum_pool = ctx.enter_context(tc.tile_pool(name="ps", bufs=8, space="PSUM"))

        f2_t = f2_pool.tile([128, 2 * HP * WP], BF16)
        f2_v = f2_t[:].rearrange("p (c y x) -> p c y x", c=2, y=HP)

        # zero the pad border once (disjoint regions)
        nc.vector.memset(f2_v[:, :, 0:PAD, :], 0.0)
        nc.vector.memset(f2_v[:, :, HP - PAD : HP, :], 0.0)
        nc.vector.memset(f2_v[:, :, PAD : HP - PAD, 0:PAD], 0.0)
        nc.vector.memset(f2_v[:, :, PAD : HP - PAD, WP - PAD : WP], 0.0)

        # interior loads, need-ordered on the sync HWDGE FIFO
        f1_tiles = []
        for g in range(NG):
            lo, hi = CHUNKS[g]
            for ch in range(2):
                nc.sync.dma_start(
                    f2_v[:, ch, PAD + lo : PAD + hi, PAD : PAD + W],
                    f2_d[:, ch, lo:hi, :],
                )
            f1_t = f1_pool.tile([128, 2 * G * W], BF16, tag="f1g")
            nc.sync.dma_start(f1_t[:], f1r_d[g])
            f1_tiles.append(f1_t)

        stage_t = stage_pool.tile([128, NG * NC_ * NMM], BF16)
        stage_r = stage_t[:].rearrange("p (t e) -> p t e", e=NMM)

        for g in range(NG):
            # f1 block layout [c, ch, cc, s, xi]: the (s, xi) weight block for
            # one (ch, cc) is contiguous, as LDWEIGHTS requires (1 free dim)
            f1_v = f1_tiles[g][:].rearrange(
                "p (c t s x) -> p c t (s x)", c=2, t=NC_, s=G
            )
            for cc in range(NC_):
                ps = psum_pool.tile([128, NMM], F32, tag="ps")
                for ch in range(2):
                    nc.tensor.matmul(
                        ps[:],
                        f1_v[:, ch, cc, :],
                        f2_v[:, ch, G * g : G * g + ROWS, A * cc : A * cc + WIN],
                        start=(ch == 0),
                        stop=(ch == 1),
                    )
                dst = stage_r[:, g * NC_ + cc, :]
                if cc % 2 == 0:
                    nc.vector.tensor_copy(dst, ps[:])
                else:
                    nc.scalar.copy(dst, ps[:])

        # per-(g, 16-partition-group) output slabs: contiguous 160-col slices
        for g in range(NG):
            for k in range(8):
                nc.sync.dma_start(
                    s1_d[16 * k : 16 * k + 16, g * NC_ : (g + 1) * NC_, :],
                    stage_r[
                        16 * k : 16 * k + 16,
                        g * NC_ : (g + 1) * NC_,
                        32 * k : 32 * k + SLAB,
                    ],
                )

    nc.compile()
    return nc


def prep_inputs(fmap1: np.ndarray, fmap2: np.ndarray):
    f1 = np.asarray(fmap1, dtype=np.float32).reshape(B, 2, 128, NG, G, NC_, A)
    # f1r[b, g, cpart, ch, cc, s, xi]
    f1r = (
        np.ascontiguousarray(f1.transpose(0, 3, 2, 1, 5, 4, 6))
        .astype(BF16_NP)
        .reshape(B, NG, 128, 2 * G * W)
    )
    f2 = np.asarray(fmap2, dtype=np.float32).reshape(B, 2, 128, H, W)
    # f2r[b, cpart, ch, y, x]
    f2r = np.ascontiguousarray(f2.transpose(0, 2, 1, 3, 4)).astype(BF16_NP)
    return f1r, f2r


def _host_gather_idx():
    y = np.arange(H)
    x = np.arange(W)
    g = y // G
    s = y % G
    cc = x // A
    xi = x % A
    p = (8 * s)[:, None] + xi[None, :]          # [H, W]
    t = (g * NC_)[:, None] + cc[None, :]        # [H, W]
    dyg = np.arange(NB) // 9
    dxg = np.arange(NB) % 9
    e = (
        ((s % 2)[None, :, None] + dyg[:, None, None]) * WIN
        + xi[None, None, :]
        + dxg[:, None, None]
    )                                            # [81, H, W]
    flat = (p[None] * (NG * NC_) + t[None]) * SLAB + e
    return flat.reshape(-1)


_FLAT_IDX = _host_gather_idx()


def finish_host(s1_all: np.ndarray) -> np.ndarray:
    s1 = np.asarray(s1_all, dtype=np.float32).reshape(B, -1)
    return s1[:, _FLAT_IDX].reshape(B, NB, H, W)


_CACHE = {}


def _get_program():
    if "p" not in _CACHE:
        _CACHE["p"] = build_program()
    return _CACHE["p"]


def run_on_cores(fmap1, fmap2, trace=False):
    nc = _get_program()
    f1r, f2r = prep_inputs(fmap1, fmap2)
    in_maps = [{"f1r": f1r[b], "f2": f2r[b]} for b in range(B)]
    res = run_bass_kernel_spmd(nc, in_maps, core_ids=list(range(B)), trace=trace)
    s1_all = np.stack([res.results[b]["s1"] for b in range(B)], axis=0)
    out = finish_host(s1_all)
    return out, res


def kernel(fmap1: np.ndarray, fmap2: np.ndarray) -> np.ndarray:
    fmap1 = np.asarray(fmap1, dtype=np.float32)
    fmap2 = np.asarray(fmap2, dtype=np.float32)
    out, _ = run_on_cores(fmap1, fmap2, trace=False)
    return out


# revision 10
# speedup vs baseline: 1.0078x; 1.0078x over previous
"""Trainium2 Bass kernel for nn_Correlation (81-displacement cost volume).

corr(b, d, y, x) = sum_c f1[b,c,y,x] * f2[b,c,y+dy,x+dx],  d = (dy+4)*9 + (dx+4)

Sharding: data-parallel over batch B=8, one batch per NeuronCore.

Per-core algorithm (all matmuls bf16, PSUM fp32):
  Tile the (y, x) output plane into blocks of G=16 y-rows x A=8 x-cols.
  For block (g, cc) the PE computes, per channel-half ch (K=128 each):
      psum[m=(s,xi), n=(row,xw)] += f1[c, y=16g+s, x=8cc+xi] *
                                    f2p[c, yp=16g+row, xp=8cc+xw]
  with s in [0,16), xi in [0,8)  (M = 128 weights = one f1 block)
  and row in [0,24), xw in [0,16) (N = 384 = the 24x16 padded f2 window).
  Entry (s,xi,row,xw) equals corr(y=16g+s, x=8cc+xi, dy=row-s, dx=xw-xi)
  - every (dy,dx) in [0,9)^2 is present.  PE cost: 4*16*2*384 = 49k cols
  (vs 157k for the per-row band scheme).
  DVE/ACT alternate evacuating psum -> bf16 stage tile [128, 64*384].
  The per-partition shear is only within each 16-partition group k
  (p = 8s+xi, k = s//2): the needed elements of tile t lie in the
  CONTIGUOUS slice [32k, 32k+160) of that tile's 384 columns, so plain
  DMA slabs (one per 16-partition group) ship them; the fine per-lane
  gather e = (s%2+dy)*16 + xi + dx happens on host.
  f2 zero-padding (4 rows/cols each side) is done on-device via memsets;
  only the real 64x128 f2 data is DMAed.
"""

import sys

sys.path.insert(0, "/opt/trn_rl_repo")

from contextlib import ExitStack

import ml_dtypes
import numpy as np

import jax

jax.config.update("jax_compilation_cache_dir", "/root/jaxcache")
jax.config.update("jax_persistent_cache_min_entry_size_bytes", 0)
jax.config.update("jax_persistent_cache_min_compile_time_secs", 0)

import concourse.bass as bass
import concourse.tile as tile
from concourse import bacc, mybir
from concourse.bass_utils import run_bass_kernel_spmd

F32 = mybir.dt.float32
BF16 = mybir.dt.bfloat16
BF16_NP = ml_dtypes.bfloat16

B = 8
C = 256
H = 64
W = 128
PAD = 4
G = 16       # y rows per block
A = 8        # x cols per block
NG = H // G  # 4 y-blocks
NC_ = W // A  # 16 x-blocks
ROWS = G + 2 * PAD   # 24 padded rows per block window
WIN = A + 2 * PAD    # 16 padded cols per block window
NMM = ROWS * WIN     # 384 psum columns per block
HP = H + 2 * PAD     # 72
WP = W + 2 * PAD     # 136
NB = 81
SLAB = (2 * PAD + 2) * WIN  # 160: contiguous slice per 16-partition group


def build_program():
    nc = bacc.Bacc("TRN2", target_bir_lowering=False, debug=False)

    f1r_d = nc.dram_tensor("f1r", [NG, 128, 2 * G * W], BF16, kind="ExternalInput").ap()
    f2_d = nc.dram_tensor("f2", [128, 2, H, W], BF16, kind="ExternalInput").ap()
    s1_d = nc.dram_tensor("s1", [128, NG * NC_, SLAB], BF16, kind="ExternalOutput").ap()

    # real-row chunks, in the order the y-blocks consume them
    # padded rows [16g, 16g+24) = real rows [16g-4, 16g+20)
    CHUNKS = [(0, 20), (20, 36), (36, 52), (52, 64)]

    with tile.TileContext(nc) as tc, ExitStack() as ctx:
        f2_pool = ctx.enter_context(tc.tile_pool(name="f2", bufs=1))
        f1_pool = ctx.enter_context(tc.tile_pool(name="f1", bufs=NG))
        stage_pool = ctx.enter_context(tc.tile_pool(name="stage", bufs=1))
        psum_pool = ctx.enter_context(tc.tile_pool(name="ps", bufs=8, space="PSUM"))

        f2_t = f2_pool.tile([128, 2 * HP * WP], BF16)
        f2_v = f2_t[:].rearrange("p (c y x) -> p c y x", c=2, y=HP)

        # zero the pad border once (disjoint regions)
        nc.vector.memset(f2_v[:, :, 0:PAD, :], 0.0)
        nc.vector.memset(f2_v[:, :, HP - PAD : HP, :], 0.0)
        nc.vector.memset(f2_v[:, :, PAD : HP - PAD, 0:PAD], 0.0)
        nc.vector.memset(f2_v[:, :, PAD : HP - PAD, WP - PAD : WP], 0.0)

        # interior loads, need-ordered; f2 on the SP HWDGE ring, f1 on the
        # ACT ring so the two streams transfer in parallel
        f1_tiles = []
        for g in range(NG):
            lo, hi = CHUNKS[g]
            for ch in range(2):
                nc.sync.dma_start(
                    f2_v[:, ch, PAD + lo : PAD + hi, PAD : PAD + W],
                    f2_d[:, ch, lo:hi, :],
                )
            f1_t = f1_pool.tile([128, 2 * G * W], BF16, tag="f1g")
            nc.scalar.dma_start(f1_t[:], f1r_d[g])
            f1_tiles.append(f1_t)

        stage_t = stage_pool.tile([128, NG * NC_ * NMM], BF16)
        stage_r = stage_t[:].rearrange("p (t e) -> p t e", e=NMM)

        for g in range(NG):
            # f1 block layout [c, ch, cc, s, xi]: the (s, xi) weight block for
            # one (ch, cc) is contiguous, as LDWEIGHTS requires (1 free dim)
            f1_v = f1_tiles[g][:].rearrange(
                "p (c t s x) -> p c t (s x)", c=2, t=NC_, s=G
            )
            for cc in range(NC_):
                ps = psum_pool.tile([128, NMM], F32, tag="ps")
                for ch in range(2):
                    nc.tensor.matmul(
                        ps[:],
                        f1_v[:, ch, cc, :],
                        f2_v[:, ch, G * g : G * g + ROWS, A * cc : A * cc + WIN],
                        start=(ch == 0),
                        stop=(ch == 1),
                    )
                dst = stage_r[:, g * NC_ + cc, :]
                if cc % 2 == 0:
                    nc.vector.tensor_copy(dst, ps[:])
                else:
                    nc.scalar.copy(dst, ps[:])

        # per-16-partition-group output slabs (contiguous 160-col slices),
        # two waves of 8 covering 2 y-blocks each, split across both rings
        for w in range(2):
            t0, t1 = 2 * w * NC_, (2 * w + 2) * NC_
            for k in range(8):
                eng = nc.sync if k % 2 == 0 else nc.scalar
                eng.dma_start(
                    s1_d[16 * k : 16 * k + 16, t0:t1, :],
                    stage_r[16 * k : 16 * k + 16, t0:t1, 32 * k : 32 * k + SLAB],
                )

    nc.compile()
    return nc


def prep_inputs(fmap1: np.ndarray, fmap2: np.ndarray):
    f1 = np.asarray(fmap1, dtype=np.float32).reshape(B, 2, 128, NG, G, NC_, A)
    # f1r[b, g, cpart, ch, cc, s, xi]
    f1r = (
        np.ascontiguousarray(f1.transpose(0, 3, 2, 1, 5, 4, 6))
        .astype(BF16_NP)
        .reshape(B, NG, 128, 2 * G * W)
    )
    f2 = np.asarray(fmap2, dtype=np.float32).reshape(B, 2, 128, H, W)
    # f2r[b, cpart, ch, y, x]
    f2r = np.ascontiguousarray(f2.transpose(0, 2, 1, 3, 4)).astype(BF16_NP)
    return f1r, f2r


def _host_gather_idx():
    y = np.arange(H)
    x = np.arange(W)
    g = y // G
    s = y % G
    cc = x // A
    xi = x % A
    p = (8 * s)[:, None] + xi[None, :]          # [H, W]
    t = (g * NC_)[:, None] + cc[None, :]        # [H, W]
    dyg = np.arange(NB) // 9
    dxg = np.arange(NB) % 9
    e = (
        ((s % 2)[None, :, None] + dyg[:, None, None]) * WIN
        + xi[None, None, :]
        + dxg[:, None, None]
    )                                            # [81, H, W]
    flat = (p[None] * (NG * NC_) + t[None]) * SLAB + e
    return flat.reshape(-1)


_FLAT_IDX = _host_gather_idx()


def finish_host(s1_all: np.ndarray) -> np.ndarray:
    s1 = np.asarray(s1_all, dtype=np.float32).reshape(B, -1)
    return s1[:, _FLAT_IDX].reshape(B, NB, H, W)


_CACHE = {}


def _get_program():
    if "p" not in _CACHE:
        _CACHE["p"] = build_program()
    return _CACHE["p"]


def run_on_cores(fmap1, fmap2, trace=False):
    nc = _get_program()
    f1r, f2r = prep_inputs(fmap1, fmap2)
    in_maps = [{"f1r": f1r[b], "f2": f2r[b]} for b in range(B)]
    res = run_bass_kernel_spmd(nc, in_maps, core_ids=list(range(B)), trace=trace)
    s1_all = np.stack([res.results[b]["s1"] for b in range(B)], axis=0)
    out = finish_host(s1_all)
    return out, res


def kernel(fmap1: np.ndarray, fmap2: np.ndarray) -> np.ndarray:
    fmap1 = np.asarray(fmap1, dtype=np.float32)
    fmap2 = np.asarray(fmap2, dtype=np.float32)
    out, _ = run_on_cores(fmap1, fmap2, trace=False)
    return out
